# revision 15
# baseline (speedup 1.0000x reference)
"""Trainium2 Bass kernel for batched masked-Kabsch RMSD (Coords2RMSD).

Contract: kernel(**inputs) takes FULL inputs (input [128, 49152] f32,
target [128, 49152] f32, num_atoms [128] i32) and returns the FULL
output [128] f32.  Internally shards batch rows across 8 NeuronCores
(16 rows per core), runs one SPMD Bass program, and gathers.

Device algorithm (per core, 16 rows):
  - Host packs 6 fp8e4m3 channels per (row, atom): (x0,x1,x2,y0,y1,
    y2), masked/zeroed beyond each row's num_atoms, in atom-transposed
    layout D[p, 192*gg + 96*t + 16*c + r] where atom a = 128*(2*gg+t)+p.
    All aux constants (selectors, masks, 1/n) ride the tail of the same
    stream as raw bytes, read back via bitcast views, so HWDGE streams
    data back-to-back with no extra descriptor generation.
  - Per 256-atom group pair gg, two accumulating PE DoubleRow matmuls:
    G[96,96] += pair^T pair and Gs[96,1] += pair^T ones.
  - Extraction: masked row-reduce of G, wide column-scatter TTs build
    p2b (bf16 for the covariance path, f32 for the E0 path), then two
    selector matmuls yield stats[16, 99] with columns pre-arranged
    (incl. signed/permuted M copies) for a wide fused epilogue.
  - Epilogue: covariance C (27 operand columns) in 3 ops, det(C) in 2,
    E0 in 2, T1 = ||C||^2 / T2 = ||C^T C||^2 in 4.  Singular value sum
    via one fixed-point step q0 = sqrt(T1 + sqrt(2)*sqrt(T1^2-T2)) plus
    reflection correction 2*sqrt(2)*(|det|-det)/(2*sqrt(T1^2-T2)):
    3 ACT Sqrt stages.  rmsd = sqrt((E0-2*sum_s)/n + 1e-8) with 1/n
    folded into the final activation's scale.  Max rel err ~1.4e-3 on
    top of the fp8 front end.
"""

import os
import sys

import numpy as np

for _p in ("/opt/trn_rl_repo", "/root/.axon_site/_ro/trn_rl_repo"):
    if os.path.isdir(_p) and _p not in sys.path:
        sys.path.insert(0, _p)

B = 128
MAX_ATOMS = 16384
NCORES = 8
ROWS = B // NCORES          # 16 rows per core
NGG = 64                    # 256-atom group pairs per row-set
CH = 6                      # channels: x0,x1,x2,y0,y1,y2
STA = CH * ROWS             # 96 data columns (x,y) per k-tile
PCOLS = 2 * STA             # 192 columns per group pair
DCOLS = NGG * PCOLS         # 12288
# DMA chunks in group pairs: sized so PE never backlogs (ratio >= 0.59)
# and the tail chunk is small.
CHUNK_GROUPS = [24, 14, 9, 7, 6, 4]
assert sum(CHUNK_GROUPS) == NGG

# aux bytes appended to d, covered by the tail chunk:
#   0:64    sel f32 [96,16] (selector; also the r'==r mask via views)
#   64:96   sel bf16 [96,16] (selector for the bf16 stats matmul)
#   96:104  rn/eps f32 [16,2]
#   104:302 masks bf16 [96,99]; 302:304 pad (keeps f32 bitcast aligned)
AUX_BYTES = 304
SEL_OFF = 0
SELB_OFF = 64
RNE_OFF = 96
MSK_OFF = 104
TAIL_DATA = CHUNK_GROUPS[-1] * PCOLS
D_W = DCOLS + AUX_BYTES

SQRT2 = 1.4142135623730951

# det(C) operand layout: det = sum_s CA[s]*CB[s]*(SC[s]*C[UC[s],2]),
# CA[s] = C[UA[s],0], CB[s] = C[UB[s],1].
UA = [0, 0, 1, 2, 1, 2]
UB = [1, 2, 0, 0, 2, 1]
UC = [2, 1, 2, 1, 0, 0]
SC = [1.0, -1.0, -1.0, 1.0, 1.0, -1.0]
U27 = [0, 0, 0, 1, 1, 1, 2, 2, 2] + UA + UB + UC
V27 = [0, 1, 2, 0, 1, 2, 0, 1, 2] + [0] * 6 + [1] * 6 + [2] * 6
S27 = [1.0] * 21 + SC

_state = {}


def _build():
    import concourse.bacc as bacc
    import concourse.mybir as mybir
    import concourse.tile as tile

    dt = mybir.dt
    AFT = mybir.ActivationFunctionType
    ALU = mybir.AluOpType
    AX = mybir.AxisListType

    nc = bacc.Bacc("TRN2", target_bir_lowering=False, debug=False)

    d_d = nc.dram_tensor("d", [128, D_W], dt.float8e4, kind="ExternalInput").ap()
    o_d = nc.dram_tensor("o", [ROWS, 1], dt.float32, kind="ExternalOutput").ap()

    with tile.TileContext(nc) as tc:
        with (
            tc.tile_pool(name="data", bufs=1) as data_pool,
            tc.tile_pool(name="small", bufs=1) as small_pool,
            tc.tile_pool(name="ep", bufs=1) as ep_pool,
            tc.tile_pool(name="psum", bufs=1, space="PSUM") as psum_pool,
        ):
            g_ps = psum_pool.tile([STA, STA], dt.float32, tag="gram")
            gs_ps = psum_pool.tile([STA, 1], dt.float32, tag="gsum")
            stats_ps = psum_pool.tile([16, 99], dt.float32, tag="stats")

            ones2 = small_pool.tile([128, 2], dt.float8e4, tag="ones")
            nc.gpsimd.memset(ones2[:], 1.0)
            ones2v = ones2[:].rearrange("p (t c) -> p t c", t=2)
            # E0 weight tile: (1x6, rn*s6pos); ones preset, tail at runtime
            wt = small_pool.tile([16, 12], dt.float32, tag="wt")
            nc.gpsimd.memset(wt[:, 0:6], 1.0)
            # warm the Sqrt act-table load off the critical path
            warm = small_pool.tile([16, 1], dt.float32, tag="warm")
            nc.scalar.activation(warm[:], ones2[0:16, 0:1], AFT.Sqrt)

            tail_tile = None
            g0 = 0
            for chunk, gpc in enumerate(CHUNK_GROUPS):
                last = chunk == len(CHUNK_GROUPS) - 1
                ccols = gpc * PCOLS + (AUX_BYTES if last else 0)
                dtile = data_pool.tile([128, ccols], dt.float8e4, tag=f"d{chunk}")
                if last:
                    tail_tile = dtile
                sl = slice(PCOLS * g0, PCOLS * g0 + ccols)
                nc.sync.dma_start(out=dtile[:], in_=d_d[:, sl])
                for gl in range(gpc):
                    gg = g0 + gl
                    base = PCOLS * gl
                    pair = dtile[:, base : base + PCOLS].rearrange(
                        "p (t c) -> p t c", t=2)
                    nc.tensor.matmul(
                        g_ps[:],
                        pair,
                        pair,
                        start=(gg == 0),
                        stop=(gg == NGG - 1),
                        perf_mode=mybir.MatmulPerfMode.DoubleRow,
                    )
                    nc.tensor.matmul(
                        gs_ps[:],
                        pair,
                        ones2v,
                        start=(gg == 0),
                        stop=(gg == NGG - 1),
                        perf_mode=mybir.MatmulPerfMode.DoubleRow,
                    )
                g0 += gpc

            # aux views into the tail chunk
            ta = TAIL_DATA
            sel = tail_tile[0:STA, ta + SEL_OFF : ta + SEL_OFF + 64].bitcast(
                dt.float32)
            selb = tail_tile[0:STA, ta + SELB_OFF : ta + SELB_OFF + 32].bitcast(
                dt.bfloat16)
            rne = tail_tile[0:16, ta + RNE_OFF : ta + RNE_OFF + 8].bitcast(
                dt.float32)
            msk = tail_tile[0:STA, ta + MSK_OFF : ta + MSK_OFF + 198].bitcast(
                dt.bfloat16)
            rn = rne[:, 0:1]
            eps = rne[:, 1:2]
            mnat = msk[:, 0:9]
            mdet = msk[:, 9:27]
            mgsb = msk[:, 27:81]
            mdiag = msk[:, 81:87]
            mgsf = msk[:, 87:99]

            # ---- stats extraction: G/Gs -> stats [16, 99] ------------------
            # stats cols: 0:9 M-nat, 9:27 M-det18, 27:54 sxR27, 54:81 syR27
            # (bf16 matmul); 81:87 diag6, 87:93 s6neg, 93:99 s6pos (f32).
            TT = nc.vector.tensor_tensor
            STT = nc.vector.scalar_tensor_tensor
            TS = nc.vector.tensor_scalar

            pmask = small_pool.tile([STA, STA], dt.float32, tag="pmask")
            TT(pmask[:].rearrange("p (c r) -> p c r", r=ROWS),
               g_ps[:].rearrange("p (c r) -> p c r", r=ROWS),
               sel.unsqueeze(1).broadcast_to([STA, CH, ROWS]), ALU.mult)
            rred = small_pool.tile([STA, 6], dt.float32, tag="rred")
            nc.vector.tensor_reduce(
                rred[:], pmask[:].rearrange("p (c r) -> p c r", r=ROWS), AX.X, ALU.add
            )
            p2b = small_pool.tile([STA, 81], dt.bfloat16, tag="p2b")
            p2f = small_pool.tile([STA, 18], dt.float32, tag="p2f")
            # M natural 9: value rred[q, 3+j] at col (i,j)
            TT(p2b[:, 0:9].rearrange("p (i j) -> p i j", j=3),
               rred[:, 3:6].unsqueeze(1).broadcast_to([STA, 3, 3]),
               mnat.rearrange("p (i j) -> p i j", j=3), ALU.mult)
            # M det blocks: value rred[q, 3+b] at col (b, s)
            TT(p2b[:, 9:27].rearrange("p (b s) -> p b s", s=6),
               rred[:, 3:6].unsqueeze(2).broadcast_to([STA, 3, 6]),
               mdet.rearrange("p (b s) -> p b s", s=6), ALU.mult)
            # gs scatter: sxR27, syR27
            TT(p2b[:, 27:81], gs_ps[:, 0:1].broadcast_to([STA, 54]), mgsb, ALU.mult)
            nc.tensor.matmul(stats_ps[:, 0:81], selb, p2b[:], start=True, stop=True)
            # diag6 + (s6neg, s6pos) in f32 for the E0 path
            TT(p2f[:, 0:6], rred[:, 0:6], mdiag, ALU.mult)
            TT(p2f[:, 6:18], gs_ps[:, 0:1].broadcast_to([STA, 12]), mgsf, ALU.mult)
            nc.tensor.matmul(stats_ps[:, 81:99], sel, p2f[:], start=True, stop=True)

            # ---------------- epilogue (per-row, 16 partitions) ------------
            _ep_ctr = [0]

            def ept(w):
                _ep_ctr[0] += 1
                nm = f"ep{_ep_ctr[0]}"
                return ep_pool.tile([16, w], dt.float32, name=nm, tag=nm)

            # sy columns PSUM->SBUF on the idle ACT engine (only O27 needs
            # a second tensor operand in SBUF; everything else reads PSUM)
            syR = ept(27)
            nc.scalar.copy(syR[:], stats_ps[:, 54:81])

            # C27: cols 0:9 natural C, 9:15 CA, 15:21 CB, 21:27 signed CC
            # C = M + (rn*sx) (x) (-sy)  (sy columns sign-folded on host)
            O27 = ept(27)
            STT(O27[:], stats_ps[:, 27:54], rn, syR[:], ALU.mult, ALU.mult)
            C27 = ept(27)
            TT(C27[:], O27[:], stats_ps[:, 0:27], ALU.add)
            C9 = C27[:, 0:9]

            # T1 = ||C||^2 ; W27/A9 -> T2 = ||C^T C||^2
            j9a = ept(9)
            T1 = ept(1)
            STT(j9a[:], C9, 1.0, C9, ALU.mult, ALU.mult, accum_out=T1[:])
            W27 = ept(27)
            w3 = W27[:].rearrange("p (i j a) -> p i j a", j=3, a=3)
            cu = C9.rearrange("p (a i) -> p i a", i=3).unsqueeze(2)
            cv = C9.rearrange("p (a j) -> p j a", j=3).unsqueeze(1)
            TT(w3, cu.broadcast_to([16, 3, 3, 3]), cv.broadcast_to([16, 3, 3, 3]),
               ALU.mult)
            A9 = ept(9)
            nc.vector.tensor_reduce(
                A9[:].rearrange("p (i j) -> p i j", j=3), w3, AX.X, ALU.add
            )
            T1sq = ept(1)
            nc.vector.tensor_scalar_mul(T1sq[:], T1[:], T1[:, 0:1])
            j9b = ept(9)
            T2n = ept(1)
            STT(j9b[:], A9[:], -1.0, A9[:], ALU.mult, ALU.mult, accum_out=T2n[:])
            # det(C); gmd = -4*sqrt(2)*min(det,0) = 2*sqrt(2)*(|det|-det)
            V6 = ept(6)
            TT(V6[:], C27[:, 9:15], C27[:, 15:21], ALU.mult)
            j6 = ept(6)
            detC = ept(1)
            STT(j6[:], V6[:], 1.0, C27[:, 21:27], ALU.mult, ALU.mult,
                accum_out=detC[:])
            gmd = ept(1)
            TS(gmd[:], detC[:], 0.0, -4.0 * SQRT2, ALU.min, ALU.mult)

            # E0 = sum(diag6) - rn*sum(s^2)
            nc.vector.tensor_scalar_mul(wt[:, 6:12], stats_ps[:, 93:99], rn)
            j12 = ept(12)
            E0 = ept(1)
            STT(j12[:], stats_ps[:, 81:93], 1.0, wt[:, 0:12], ALU.mult, ALU.mult,
                accum_out=E0[:])

            # sqe = sqrt(T1^2 - T2); q0 = sqrt(T1 + sqrt(2)*sqe)
            sqe = ept(1)
            nc.scalar.activation(sqe[:], T2n[:], AFT.Sqrt, bias=T1sq[:, 0:1])
            q0 = ept(1)
            nc.scalar.activation(q0[:], sqe[:], AFT.Sqrt, bias=T1[:, 0:1],
                                 scale=SQRT2)
            # correction terms computed while ACT works on q0
            isqe = ept(1)
            nc.vector.reciprocal(isqe[:], sqe[:])
            EQ = ept(1)
            STT(EQ[:], q0[:], -2.0, E0[:], ALU.mult, ALU.add)
            # t11 = E0 - 2*q0 + 2*sqrt(2)*(|det|-det)/sqe
            t11 = ept(1)
            STT(t11[:], gmd[:], isqe[:, 0:1], EQ[:], ALU.mult, ALU.add)
            rmsd = ept(1)
            nc.scalar.activation(rmsd[:], t11[:], AFT.Sqrt, bias=eps, scale=rn)
            nc.sync.dma_start(out=o_d, in_=rmsd[:])

    nc.compile()
    return nc


def _host_pack(input, target, num_atoms):
    """[NCORES, 128, D_W] fp8 bytes: packed coords + aux tail."""
    import ml_dtypes

    fp8 = ml_dtypes.float8_e4m3
    x3 = input.reshape(B, MAX_ATOMS, 3)
    y3 = target.reshape(B, MAX_ATOMS, 3)
    mask = np.arange(MAX_ATOMS)[None, :] < num_atoms[:, None]
    Z = np.empty((B, MAX_ATOMS, CH), dtype=fp8)
    Z[:, :, 0:3] = np.where(mask[..., None], x3, 0.0).astype(fp8)
    Z[:, :, 3:6] = np.where(mask[..., None], y3, 0.0).astype(fp8)
    # [core, r, gg, t, p, c] -> [core, p, gg, t, c, r]
    Zt = Z.reshape(NCORES, ROWS, NGG, 2, 128, CH).transpose(0, 4, 2, 3, 5, 1)
    D = np.empty((NCORES, 128, D_W), dtype=np.uint8)
    D[:, :, 0:DCOLS] = (
        np.ascontiguousarray(Zt).reshape(NCORES, 128, DCOLS).view(np.uint8)
    )
    D[:, :, DCOLS:] = _host_aux_tail(num_atoms)
    return D


def _host_aux_tail(num_atoms):
    """[NCORES, 128, AUX_BYTES] raw aux bytes (sel, selb, rn/eps, masks)."""
    import ml_dtypes

    bf16 = ml_dtypes.bfloat16
    q = np.arange(STA)
    r_of_q = q % ROWS
    ci_of_q = q // ROWS

    sel = np.zeros((128, 16), dtype=np.float32)
    sel[q, r_of_q] = 1.0
    selb = np.zeros((128, 16), dtype=bf16)
    selb[q, r_of_q] = 1.0

    masks = np.zeros((128, 99), dtype=bf16)
    for i in range(3):
        for j in range(3):
            masks[q, 3 * i + j] = (ci_of_q == i)
    for s in range(6):
        masks[q, 9 + s] = (ci_of_q == UA[s])
        masks[q, 15 + s] = (ci_of_q == UB[s])
        masks[q, 21 + s] = SC[s] * (ci_of_q == UC[s])
    for s in range(27):
        masks[q, 27 + s] = 1.0 * (ci_of_q == U27[s])            # sxR27
        masks[q, 54 + s] = -S27[s] * (ci_of_q == 3 + V27[s])    # syR27
    for c in range(6):
        masks[q, 81 + c] = (ci_of_q == c)          # diag6
        masks[q, 87 + c] = -1.0 * (ci_of_q == c)   # s6neg
        masks[q, 93 + c] = 1.0 * (ci_of_q == c)    # s6pos

    aux = np.zeros((NCORES, 128, AUX_BYTES), dtype=np.uint8)
    for c in range(NCORES):
        aux[c, :, SEL_OFF : SEL_OFF + 64] = sel.view(np.uint8)
        aux[c, :, SELB_OFF : SELB_OFF + 32] = selb.view(np.uint8)
        rne = np.zeros((16, 2), dtype=np.float32)
        rne[:, 0] = 1.0 / num_atoms[c * ROWS : (c + 1) * ROWS].astype(np.float32)
        rne[:, 1] = 1e-8
        aux[c, 0:16, RNE_OFF : RNE_OFF + 8] = rne.view(np.uint8)
        aux[c, :, MSK_OFF : MSK_OFF + 198] = masks.view(np.uint8)
    return aux


def kernel(input, target, num_atoms):
    from concourse.bass_utils import run_bass_kernel_spmd

    if "nc" not in _state:
        _state["nc"] = _build()
    nc = _state["nc"]

    input = np.ascontiguousarray(np.asarray(input), dtype=np.float32)
    target = np.ascontiguousarray(np.asarray(target), dtype=np.float32)
    num_atoms = np.asarray(num_atoms)

    D = _host_pack(input, target, num_atoms)

    in_maps = [{"d": D[c]} for c in range(NCORES)]
    res = run_bass_kernel_spmd(nc, in_maps, core_ids=list(range(NCORES)))
    out = np.concatenate([np.asarray(r["o"]).reshape(ROWS) for r in res.results])
    return out.astype(np.float32)


# revision 16
# speedup vs baseline: 1.0441x; 1.0441x over previous
"""Trainium2 Bass kernel for batched masked-Kabsch RMSD (Coords2RMSD).

Contract: kernel(**inputs) takes FULL inputs (input [128, 49152] f32,
target [128, 49152] f32, num_atoms [128] i32) and returns the FULL
output [128] f32.  Internally shards batch rows across 8 NeuronCores
(16 rows per core), runs one SPMD Bass program, and gathers.

Device algorithm (per core, 16 rows):
  - Host packs 6 fp8e4m3 channels per (row, atom): (x0,x1,x2,y0,y1,
    y2), masked/zeroed beyond each row's num_atoms, in atom-transposed
    layout D[p, 192*gg + 96*t + 16*c + r] where atom a = 128*(2*gg+t)+p.
    All aux constants (selectors, masks, 1/n) ride the tail of the same
    stream as raw bytes, read back via bitcast views, so HWDGE streams
    data back-to-back with no extra descriptor generation.
  - Per 256-atom group pair gg, two accumulating PE DoubleRow matmuls:
    G[96,96] += pair^T pair and Gs[96,1] += pair^T ones.
  - Extraction: masked row-reduce of G, wide column-scatter TTs build
    p2b (bf16 for the covariance path, f32 for the E0 path), then two
    selector matmuls yield stats[16, 99] with columns pre-arranged
    (incl. signed/permuted M copies) for a wide fused epilogue.
  - Epilogue: covariance C (27 operand columns) in 3 ops, det(C) in 2,
    E0 in 2, T1 = ||C||^2 / T2 = ||C^T C||^2 in 4.  Singular value sum
    via one fixed-point step q0 = sqrt(T1 + sqrt(2)*sqrt(T1^2-T2)) plus
    reflection correction 2*sqrt(2)*(|det|-det)/(2*sqrt(T1^2-T2)):
    3 ACT Sqrt stages.  rmsd = sqrt((E0-2*sum_s)/n + 1e-8) with 1/n
    folded into the final activation's scale.  Max rel err ~1.4e-3 on
    top of the fp8 front end.
"""

import os
import sys

import numpy as np

for _p in ("/opt/trn_rl_repo", "/root/.axon_site/_ro/trn_rl_repo"):
    if os.path.isdir(_p) and _p not in sys.path:
        sys.path.insert(0, _p)

B = 128
MAX_ATOMS = 16384
NCORES = 8
ROWS = B // NCORES          # 16 rows per core
NGG = 64                    # 256-atom group pairs per row-set
CH = 6                      # channels: x0,x1,x2,y0,y1,y2
STA = CH * ROWS             # 96 data columns (x,y) per k-tile
PCOLS = 2 * STA             # 192 columns per group pair
DCOLS = NGG * PCOLS         # 12288
# DMA chunks in group pairs: sized so PE never backlogs (ratio >= 0.59)
# and the tail chunk is small.
CHUNK_GROUPS = [24, 14, 9, 7, 6, 4]
assert sum(CHUNK_GROUPS) == NGG

# aux bytes appended to d, covered by the tail chunk:
#   0:64    sel f32 [96,16] (selector; also the r'==r mask via views)
#   64:96   sel bf16 [96,16] (selector for the bf16 stats matmul)
#   96:104  rn/eps f32 [16,2]
#   104:302 masks bf16 [96,99]; 302:304 pad (keeps f32 bitcast aligned)
AUX_BYTES = 304
SEL_OFF = 0
SELB_OFF = 64
RNE_OFF = 96
MSK_OFF = 104
TAIL_DATA = CHUNK_GROUPS[-1] * PCOLS
D_W = DCOLS + AUX_BYTES

SQRT2 = 1.4142135623730951

# det(C) operand layout: det = sum_s CA[s]*CB[s]*(SC[s]*C[UC[s],2]),
# CA[s] = C[UA[s],0], CB[s] = C[UB[s],1].
UA = [0, 0, 1, 2, 1, 2]
UB = [1, 2, 0, 0, 2, 1]
UC = [2, 1, 2, 1, 0, 0]
SC = [1.0, -1.0, -1.0, 1.0, 1.0, -1.0]
U27 = [0, 0, 0, 1, 1, 1, 2, 2, 2] + UA + UB + UC
V27 = [0, 1, 2, 0, 1, 2, 0, 1, 2] + [0] * 6 + [1] * 6 + [2] * 6
S27 = [1.0] * 21 + SC

_state = {}


def _build():
    import concourse.bacc as bacc
    import concourse.mybir as mybir
    import concourse.tile as tile

    dt = mybir.dt
    AFT = mybir.ActivationFunctionType
    ALU = mybir.AluOpType
    AX = mybir.AxisListType

    nc = bacc.Bacc("TRN2", target_bir_lowering=False, debug=False)

    d_d = nc.dram_tensor("d", [128, D_W], dt.float8e4, kind="ExternalInput").ap()
    o_d = nc.dram_tensor("o", [ROWS, 1], dt.float32, kind="ExternalOutput").ap()

    with tile.TileContext(nc) as tc:
        with (
            tc.tile_pool(name="data", bufs=1) as data_pool,
            tc.tile_pool(name="small", bufs=1) as small_pool,
            tc.tile_pool(name="ep", bufs=1) as ep_pool,
            tc.tile_pool(name="psum", bufs=1, space="PSUM") as psum_pool,
        ):
            g_ps = psum_pool.tile([STA, STA], dt.float32, tag="gram")
            gs_ps = psum_pool.tile([STA, 1], dt.float32, tag="gsum")
            stats_ps = psum_pool.tile([16, 99], dt.float32, tag="stats")

            ones2 = small_pool.tile([128, 2], dt.float8e4, tag="ones")
            nc.gpsimd.memset(ones2[:], 1.0)
            ones2v = ones2[:].rearrange("p (t c) -> p t c", t=2)
            # E0 weight tile: (1x6, rn*s6pos); ones preset, tail at runtime
            wt = small_pool.tile([16, 12], dt.float32, tag="wt")
            nc.gpsimd.memset(wt[:, 0:6], 1.0)
            # warm the Sqrt act-table load off the critical path
            warm = small_pool.tile([16, 1], dt.float32, tag="warm")
            nc.scalar.activation(warm[:], ones2[0:16, 0:1], AFT.Sqrt)

            tail_tile = None
            g0 = 0
            for chunk, gpc in enumerate(CHUNK_GROUPS):
                last = chunk == len(CHUNK_GROUPS) - 1
                ccols = gpc * PCOLS + (AUX_BYTES if last else 0)
                dtile = data_pool.tile([128, ccols], dt.float8e4, tag=f"d{chunk}")
                if last:
                    tail_tile = dtile
                sl = slice(PCOLS * g0, PCOLS * g0 + ccols)
                nc.sync.dma_start(out=dtile[:], in_=d_d[:, sl])
                for gl in range(gpc):
                    gg = g0 + gl
                    base = PCOLS * gl
                    pair = dtile[:, base : base + PCOLS].rearrange(
                        "p (t c) -> p t c", t=2)
                    nc.tensor.matmul(
                        g_ps[:],
                        pair,
                        pair,
                        start=(gg == 0),
                        stop=(gg == NGG - 1),
                        perf_mode=mybir.MatmulPerfMode.DoubleRow,
                    )
                    nc.tensor.matmul(
                        gs_ps[:],
                        pair,
                        ones2v,
                        start=(gg == 0),
                        stop=(gg == NGG - 1),
                        perf_mode=mybir.MatmulPerfMode.DoubleRow,
                    )
                g0 += gpc

            # aux views into the tail chunk
            ta = TAIL_DATA
            sel = tail_tile[0:STA, ta + SEL_OFF : ta + SEL_OFF + 64].bitcast(
                dt.float32)
            selb = tail_tile[0:STA, ta + SELB_OFF : ta + SELB_OFF + 32].bitcast(
                dt.bfloat16)
            rne = tail_tile[0:16, ta + RNE_OFF : ta + RNE_OFF + 8].bitcast(
                dt.float32)
            msk = tail_tile[0:STA, ta + MSK_OFF : ta + MSK_OFF + 198].bitcast(
                dt.bfloat16)
            rn = rne[:, 0:1]
            eps = rne[:, 1:2]
            mnat = msk[:, 0:9]
            mdet = msk[:, 9:27]
            mgsb = msk[:, 27:81]
            mdiag = msk[:, 81:87]
            mgsf = msk[:, 87:99]

            # ---- stats extraction: G/Gs -> stats [16, 99] ------------------
            # stats cols: 0:9 M-nat, 9:27 M-det18, 27:54 sxR27, 54:81 syR27
            # (bf16 matmul); 81:87 diag6, 87:93 s6neg, 93:99 s6pos (f32).
            TT = nc.vector.tensor_tensor
            STT = nc.vector.scalar_tensor_tensor
            TS = nc.vector.tensor_scalar

            pmask = small_pool.tile([STA, STA], dt.float32, tag="pmask")
            TT(pmask[:].rearrange("p (c r) -> p c r", r=ROWS),
               g_ps[:].rearrange("p (c r) -> p c r", r=ROWS),
               sel.unsqueeze(1).broadcast_to([STA, CH, ROWS]), ALU.mult)
            rred = small_pool.tile([STA, 6], dt.float32, tag="rred")
            nc.vector.tensor_reduce(
                rred[:], pmask[:].rearrange("p (c r) -> p c r", r=ROWS), AX.X, ALU.add
            )
            p2b = small_pool.tile([STA, 81], dt.bfloat16, tag="p2b")
            p2f = small_pool.tile([STA, 18], dt.float32, tag="p2f")
            # M natural 9: value rred[q, 3+j] at col (i,j)
            TT(p2b[:, 0:9].rearrange("p (i j) -> p i j", j=3),
               rred[:, 3:6].unsqueeze(1).broadcast_to([STA, 3, 3]),
               mnat.rearrange("p (i j) -> p i j", j=3), ALU.mult)
            # M det blocks: value rred[q, 3+b] at col (b, s)
            TT(p2b[:, 9:27].rearrange("p (b s) -> p b s", s=6),
               rred[:, 3:6].unsqueeze(2).broadcast_to([STA, 3, 6]),
               mdet.rearrange("p (b s) -> p b s", s=6), ALU.mult)
            # gs scatter: sxR27, syR27
            TT(p2b[:, 27:81], gs_ps[:, 0:1].broadcast_to([STA, 54]), mgsb, ALU.mult)
            nc.tensor.matmul(stats_ps[:, 0:81], selb, p2b[:], start=True, stop=True)
            # diag6 + (s6neg, s6pos) in f32 for the E0 path
            TT(p2f[:, 0:6], rred[:, 0:6], mdiag, ALU.mult)
            TT(p2f[:, 6:18], gs_ps[:, 0:1].broadcast_to([STA, 12]), mgsf, ALU.mult)
            nc.tensor.matmul(stats_ps[:, 81:99], sel, p2f[:], start=True, stop=True)

            # ---------------- epilogue (per-row, 16 partitions) ------------
            _ep_ctr = [0]

            def ept(w):
                _ep_ctr[0] += 1
                nm = f"ep{_ep_ctr[0]}"
                return ep_pool.tile([16, w], dt.float32, name=nm, tag=nm)

            # one PSUM->SBUF hop; all epilogue reads are cheap SBUF edges
            stats = ept(99)
            nc.vector.tensor_scalar_mul(stats[:], stats_ps[:], 1.0)

            # C27: cols 0:9 natural C, 9:15 CA, 15:21 CB, 21:27 signed CC
            # C = M + (rn*sx) (x) (-sy)  (sy columns sign-folded on host)
            O27 = ept(27)
            STT(O27[:], stats[:, 27:54], rn, stats[:, 54:81], ALU.mult, ALU.mult)
            C27 = ept(27)
            TT(C27[:], O27[:], stats[:, 0:27], ALU.add)
            C9 = C27[:, 0:9]

            # T1 = ||C||^2 ; W27/A9 -> T2 = ||C^T C||^2
            j9a = ept(9)
            T1 = ept(1)
            STT(j9a[:], C9, 1.0, C9, ALU.mult, ALU.mult, accum_out=T1[:])
            W27 = ept(27)
            w3 = W27[:].rearrange("p (i j a) -> p i j a", j=3, a=3)
            cu = C9.rearrange("p (a i) -> p i a", i=3).unsqueeze(2)
            cv = C9.rearrange("p (a j) -> p j a", j=3).unsqueeze(1)
            TT(w3, cu.broadcast_to([16, 3, 3, 3]), cv.broadcast_to([16, 3, 3, 3]),
               ALU.mult)
            A9 = ept(9)
            nc.vector.tensor_reduce(
                A9[:].rearrange("p (i j) -> p i j", j=3), w3, AX.X, ALU.add
            )
            T1sq = ept(1)
            nc.vector.tensor_scalar_mul(T1sq[:], T1[:], T1[:, 0:1])
            j9b = ept(9)
            T2n = ept(1)
            STT(j9b[:], A9[:], -1.0, A9[:], ALU.mult, ALU.mult, accum_out=T2n[:])
            # det(C); gmd = -4*sqrt(2)*min(det,0) = 2*sqrt(2)*(|det|-det)
            V6 = ept(6)
            TT(V6[:], C27[:, 9:15], C27[:, 15:21], ALU.mult)
            j6 = ept(6)
            detC = ept(1)
            STT(j6[:], V6[:], 1.0, C27[:, 21:27], ALU.mult, ALU.mult,
                accum_out=detC[:])
            gmd = ept(1)
            TS(gmd[:], detC[:], 0.0, -4.0 * SQRT2, ALU.min, ALU.mult)

            # E0 = sum(diag6) - rn*sum(s^2)
            nc.vector.tensor_scalar_mul(wt[:, 6:12], stats[:, 93:99], rn)
            j12 = ept(12)
            E0 = ept(1)
            STT(j12[:], stats[:, 81:93], 1.0, wt[:, 0:12], ALU.mult, ALU.mult,
                accum_out=E0[:])

            # sqe = sqrt(T1^2 - T2); q0 = sqrt(T1 + sqrt(2)*sqe)
            sqe = ept(1)
            nc.scalar.activation(sqe[:], T2n[:], AFT.Sqrt, bias=T1sq[:, 0:1])
            q0 = ept(1)
            nc.scalar.activation(q0[:], sqe[:], AFT.Sqrt, bias=T1[:, 0:1],
                                 scale=SQRT2)
            # correction terms computed while ACT works on q0
            isqe = ept(1)
            nc.vector.reciprocal(isqe[:], sqe[:])
            EQ = ept(1)
            STT(EQ[:], q0[:], -2.0, E0[:], ALU.mult, ALU.add)
            # t11 = E0 - 2*q0 + 2*sqrt(2)*(|det|-det)/sqe
            t11 = ept(1)
            STT(t11[:], gmd[:], isqe[:, 0:1], EQ[:], ALU.mult, ALU.add)
            rmsd = ept(1)
            nc.scalar.activation(rmsd[:], t11[:], AFT.Sqrt, bias=eps, scale=rn)
            nc.sync.dma_start(out=o_d, in_=rmsd[:])

    nc.compile()
    return nc


def _host_pack(input, target, num_atoms):
    """[NCORES, 128, D_W] fp8 bytes: packed coords + aux tail."""
    import ml_dtypes

    fp8 = ml_dtypes.float8_e4m3
    x3 = input.reshape(B, MAX_ATOMS, 3)
    y3 = target.reshape(B, MAX_ATOMS, 3)
    mask = np.arange(MAX_ATOMS)[None, :] < num_atoms[:, None]
    Z = np.empty((B, MAX_ATOMS, CH), dtype=fp8)
    Z[:, :, 0:3] = np.where(mask[..., None], x3, 0.0).astype(fp8)
    Z[:, :, 3:6] = np.where(mask[..., None], y3, 0.0).astype(fp8)
    # [core, r, gg, t, p, c] -> [core, p, gg, t, c, r]
    Zt = Z.reshape(NCORES, ROWS, NGG, 2, 128, CH).transpose(0, 4, 2, 3, 5, 1)
    D = np.empty((NCORES, 128, D_W), dtype=np.uint8)
    D[:, :, 0:DCOLS] = (
        np.ascontiguousarray(Zt).reshape(NCORES, 128, DCOLS).view(np.uint8)
    )
    D[:, :, DCOLS:] = _host_aux_tail(num_atoms)
    return D


def _host_aux_tail(num_atoms):
    """[NCORES, 128, AUX_BYTES] raw aux bytes (sel, selb, rn/eps, masks)."""
    import ml_dtypes

    bf16 = ml_dtypes.bfloat16
    q = np.arange(STA)
    r_of_q = q % ROWS
    ci_of_q = q // ROWS

    sel = np.zeros((128, 16), dtype=np.float32)
    sel[q, r_of_q] = 1.0
    selb = np.zeros((128, 16), dtype=bf16)
    selb[q, r_of_q] = 1.0

    masks = np.zeros((128, 99), dtype=bf16)
    for i in range(3):
        for j in range(3):
            masks[q, 3 * i + j] = (ci_of_q == i)
    for s in range(6):
        masks[q, 9 + s] = (ci_of_q == UA[s])
        masks[q, 15 + s] = (ci_of_q == UB[s])
        masks[q, 21 + s] = SC[s] * (ci_of_q == UC[s])
    for s in range(27):
        masks[q, 27 + s] = 1.0 * (ci_of_q == U27[s])            # sxR27
        masks[q, 54 + s] = -S27[s] * (ci_of_q == 3 + V27[s])    # syR27
    for c in range(6):
        masks[q, 81 + c] = (ci_of_q == c)          # diag6
        masks[q, 87 + c] = -1.0 * (ci_of_q == c)   # s6neg
        masks[q, 93 + c] = 1.0 * (ci_of_q == c)    # s6pos

    aux = np.zeros((NCORES, 128, AUX_BYTES), dtype=np.uint8)
    for c in range(NCORES):
        aux[c, :, SEL_OFF : SEL_OFF + 64] = sel.view(np.uint8)
        aux[c, :, SELB_OFF : SELB_OFF + 32] = selb.view(np.uint8)
        rne = np.zeros((16, 2), dtype=np.float32)
        rne[:, 0] = 1.0 / num_atoms[c * ROWS : (c + 1) * ROWS].astype(np.float32)
        rne[:, 1] = 1e-8
        aux[c, 0:16, RNE_OFF : RNE_OFF + 8] = rne.view(np.uint8)
        aux[c, :, MSK_OFF : MSK_OFF + 198] = masks.view(np.uint8)
    return aux


def kernel(input, target, num_atoms):
    from concourse.bass_utils import run_bass_kernel_spmd

    if "nc" not in _state:
        _state["nc"] = _build()
    nc = _state["nc"]

    input = np.ascontiguousarray(np.asarray(input), dtype=np.float32)
    target = np.ascontiguousarray(np.asarray(target), dtype=np.float32)
    num_atoms = np.asarray(num_atoms)

    D = _host_pack(input, target, num_atoms)

    in_maps = [{"d": D[c]} for c in range(NCORES)]
    res = run_bass_kernel_spmd(nc, in_maps, core_ids=list(range(NCORES)))
    out = np.concatenate([np.asarray(r["o"]).reshape(ROWS) for r in res.results])
    return out.astype(np.float32)
